# revision 30
# baseline (speedup 1.0000x reference)
"""Causal self-attention (B=2, T=4096, C=768, H=12, D=64) on 8 Trainium2 cores.

Sharding: 2 batches x 4 head-groups (3 heads each). Per core:
  - qkv projection for its 3 heads, computed in transposed layout [dim, T]
  - causal attention per head: S^T blocks (bf16 matmuls), exp on the
    Activation engine straight to fp8e4m3 with a -1 shift (cancels in the
    softmax normalization), causal masking by zeroing exp output on the
    GpSimd engine, and P@V as fp8 DoubleRow matmuls against a hi/lo fp8
    split of V (error ~bf16 level, half the PE cost of bf16)
  - row-parallel output projection partial [T, C] (contraction packed
    128+65 rows -> 2 matmuls per 384-wide chunk)
  - ReduceScatter(add) over the 4 cores of the same batch -> [T/4, C] slice

Host pre-casts x and the weight slices to bf16. Host gathers the 8
[1024, 768] slices into [2, 4096, 768].
"""

import sys

sys.path.insert(0, "/opt/trn_rl_repo")

import numpy as np
import ml_dtypes

import concourse.bass as bass
import concourse.tile as tile
from concourse import bacc, mybir
from concourse.bass import ds
from concourse.bass_utils import run_bass_kernel_spmd
from concourse.masks import make_identity

T = 4096
C = 768
D = 64
NCORES = 8
G = 4  # cores per batch (head-groups)
HPC = 3  # heads per core
TSL = T // G  # output token slice per core
QC = 512  # q-chunk (free dim of S^T matmuls)
NQC = T // QC
F32 = mybir.dt.float32
BF16 = mybir.dt.bfloat16
F8 = mybir.dt.float8e4
U16 = mybir.dt.uint16
FX = mybir.ActivationFunctionType
AL = mybir.AluOpType
DR = mybir.MatmulPerfMode.DoubleRow

SHIFT = -1.0  # exp(s/8 - 1): keeps fp8 P in range; cancels in softmax
WSC = 16.0  # fp8 scale for Wqkv; qkvT carries 16x values, folded back in norm


def _body(ctx, tc, collective=True):
    nc = tc.nc
    mm = nc.tensor.matmul
    # x and Wqkv ship as fp8 hi/lo pairs packed into uint16 (hi in the low
    # byte) so the 2-byte DMA transpose works; 3 DoubleRow chains
    # (hi*Whi + hi*Wlo + lo*Whi) recover ~bf16 accuracy at 3/4 the PE cost
    xb = nc.dram_tensor("xb", [T, C], U16, kind="ExternalInput").ap()
    wch = nc.dram_tensor("wch", [C, 576], F8, kind="ExternalInput").ap()
    wcl = nc.dram_tensor("wcl", [C, 576], F8, kind="ExternalInput").ap()
    bc = nc.dram_tensor("bc", [576], F32, kind="ExternalInput").ap()
    wp = nc.dram_tensor("wp", [193, C], BF16, kind="ExternalInput").ap()
    outp = nc.dram_tensor("outp", [TSL, C], F32, kind="ExternalOutput").ap()
    partial = nc.dram_tensor("partial", [T, C], F32).ap()
    rsout = nc.dram_tensor("rsout", [TSL, C], F32).ap()

    cp = ctx.enter_context(tc.tile_pool(name="consts", bufs=1))
    mp = ctx.enter_context(tc.tile_pool(name="main", bufs=1))

    onesT = cp.tile([65, 64], BF16)
    nc.gpsimd.memset(onesT[:], 1.0 / WSC)  # folds the W fp8 scale back out
    sh = cp.tile([128, 1], F32)
    nc.gpsimd.memset(sh[:], SHIFT)
    bcol = cp.tile([128, 5], F32)
    wpA = cp.tile([128, C], BF16)
    wpB = cp.tile([65, C], BF16)

    # qkvT partition-tiles (columns of wc, order fixed host-side):
    #   m=0: [q_h0 | q_h1]   m=1: [k_h0 | k_h1]   m=2: [v_h0 | v_h1]
    #   m=3: [q_h2 | v_h2]   m=4: [k_h2 | -]
    xT = mp.tile([128, 6, T], U16)  # packed fp8 hi/lo pairs, transposed
    qkvT = mp.tile([128, 5, T], BF16)
    # V^T hi/lo fp8 split, per 128-token tile: [h0 d0..63|1][h1 ...|1][h2 ...|1]
    vhi = mp.tile([128, T // 128, 3 * 128], F8)
    vlo = mp.tile([128, T // 128, 3 * 128], F8)
    for hh in range(3):
        # col 64 = ones (softmax denominator row); cols 65-127 zero padding
        # (DoubleRow needs the stationary free dim 2x128)
        nc.gpsimd.memset(
            vhi[:].rearrange("p t (h c) -> p t h c", c=128)[:, :, hh, 64:65], 1.0
        )
        nc.gpsimd.memset(
            vhi[:].rearrange("p t (h c) -> p t h c", c=128)[:, :, hh, 65:128], 0.0
        )
        nc.gpsimd.memset(
            vlo[:].rearrange("p t (h c) -> p t h c", c=128)[:, :, hh, 64:128], 0.0
        )
    ytA = mp.tile([128, T], BF16)  # h0 rows 0-63, h1 rows 64-127
    ytB = mp.tile([65, T], BF16)  # h2 rows 0-63, row 64 = ones (bias row)
    nc.gpsimd.memset(ytB[64:65, :], 1.0)

    qT = [qkvT[0:64, 0], qkvT[64:128, 0], qkvT[0:64, 3]]
    kT = [qkvT[0:64, 1], qkvT[64:128, 1], qkvT[0:64, 4]]
    msizes = [128, 128, 128, 128, 64]

    # PSUM budget (8 banks): ps2(2x2) + ya(2) + mm(2) = 8
    with (
        tc.tile_pool(name="wst", bufs=1) as wstp,
        tc.tile_pool(name="ex", bufs=8) as exp_,
        tc.tile_pool(name="rd", bufs=2) as rdp,
        tc.tile_pool(name="yc", bufs=2) as ycp,
        tc.tile_pool(name="prt", bufs=3) as prtp,
        tc.tile_pool(name="vstg", bufs=4) as vstgp,
        tc.tile_pool(name="ps2", bufs=3, space="PSUM") as ps2p,
        tc.tile_pool(name="ya", bufs=1, space="PSUM") as yap,
        tc.tile_pool(name="mmp", bufs=1, space="PSUM") as mmp,
    ):
        wsth = wstp.tile([128, 6, 576], F8)
        wstl = wstp.tile([128, 6, 576], F8)
        wchr = wch.rearrange("(kc p) d -> p kc d", p=128)
        wclr = wcl.rearrange("(kc p) d -> p kc d", p=128)
        # fp8 views of the packed xT: [p, kc, two, t] with two=0 -> hi
        xv = xT[:].bitcast(F8).rearrange("p k (t two) -> p k two t", two=2)

        def xT_load(t0, t1):
            # one transpose DMA per 128-col chunk covering tokens [t0, t1)
            for kc in range(6):
                nc.sync.dma_start(
                    xT[:, kc, t0:t1],
                    xb[t0:t1, :][:, ds(128 * kc, 128)],
                    transpose=True,
                )

        # ---- phase-1 sub-units (emitted as fillers between attention pairs)
        def qkv_mtile(nb, m):
            msz = msizes[m]
            psq = mmp.tile([128, QC], F32, tag="mm")
            chains = [(wsth, 0), (wstl, 0), (wsth, 1)]  # (W half, x half)
            for ci, (wt, xh) in enumerate(chains):
                for g3 in range(3):
                    mm(
                        psq[0:msz, :],
                        wt[:, ds(2 * g3, 2), ds(128 * m, msz)],
                        xv[:, ds(2 * g3, 2), ds(xh, 1), ds(QC * nb, QC)],
                        start=(ci == 0 and g3 == 0),
                        stop=(ci == 2 and g3 == 2),
                        perf_mode=DR,
                        skip_group_check=True,
                    )
            nc.vector.tensor_scalar_add(
                qkvT[0:msz, m, ds(QC * nb, QC)],
                psq[0:msz, :],
                bcol[0:msz, m : m + 1],
            )

        def v_tile(tt):
            # v^T tiles via SBUF->SBUF DMA transpose (keeps PE free), then
            # hi/lo fp8 split on DVE
            vstg = vstgp.tile([128, 192], BF16, tag="vstg", name=f"vstg_{tt}")
            nc.sync.dma_start(vstg[:, 0:128], qkvT[:, 2, ds(128 * tt, 128)],
                              transpose=True)
            nc.sync.dma_start(vstg[:, 128:192],
                              qkvT[64:128, 3, ds(128 * tt, 128)],
                              transpose=True)
            hview = vhi[:, tt, :].rearrange("p (h c) -> p h c", c=128)[:, 0:2, 0:64]
            lview = vlo[:, tt, :].rearrange("p (h c) -> p h c", c=128)[:, 0:2, 0:64]
            pview = vstg[:, 0:128].rearrange("p (h c) -> p h c", c=64)
            nc.vector.tensor_copy(hview, pview)
            nc.vector.tensor_sub(lview, pview, hview)
            nc.vector.tensor_copy(vhi[:, tt, 256:320], vstg[:, 128:192])
            nc.vector.tensor_sub(vlo[:, tt, 256:320], vstg[:, 128:192],
                                 vhi[:, tt, 256:320])

        def proj_tile(tt, end=False):
            prt = prtp.tile([128, C], F32)
            for nn in range(2):
                if end:
                    pst = ps2p.tile([128, 2, QC], F32, tag="ps2",
                                    name=f"pse_{tt}_{nn}")
                    psp = pst[:, 0, :]
                else:
                    psp = mmp.tile([128, QC], F32, tag="mm")
                mm(psp[:, 0:384], ytA[:, ds(128 * tt, 128)],
                   wpA[:, ds(384 * nn, 384)], start=True, stop=False)
                mm(psp[:, 0:384], ytB[:, ds(128 * tt, 128)],
                   wpB[:, ds(384 * nn, 384)], start=False, stop=True)
                nc.vector.tensor_copy(prt[:, ds(384 * nn, 384)], psp[:, 0:384])
            nc.sync.dma_start(partial[ds(128 * tt, 128), :], prt[:])
            if not collective and tt < TSL // 128:
                nc.sync.dma_start(outp[ds(128 * tt, 128), :], prt[:])

        # ---- filler queue machinery ----
        fillers = []  # list of (pe_cost_ns, closure)
        # proj work deferred into the late, Act-bound q-chunks
        PROJ_SCHED = {3: [0], 4: [1, 3], 5: [2, 4], 6: [5], 7: [6]}

        def push_block_fillers(qc):
            if qc + 1 < NQC:
                nb = qc + 1
                for m in range(5):
                    fillers.append((1400, lambda nb=nb, m=m: qkv_mtile(nb, m)))
                for tt in range(4 * nb, 4 * nb + 4):
                    fillers.append((0, lambda tt=tt: v_tile(tt)))
            for pq in PROJ_SCHED.get(qc, []):
                for tt in range(4 * pq, 4 * pq + 4):
                    fillers.append((700, lambda tt=tt: proj_tile(tt)))

        # ---- attention pair-units ----
        units = []
        for qc in range(NQC):
            for h in range(HPC):
                ngr = 2 * qc + 2
                for g in range(ngr):
                    units.append((qc, h, g))

        ya_tiles = {}

        def emit_scores(u):
            qc, h, g = u
            diagB = g == 2 * qc + 1
            ps2 = ps2p.tile([128, 2, QC], F32, tag="ps2")
            for i in range(2):
                kt = 2 * g + i
                if diagB:
                    mm(ps2[:, i, 256:512], kT[h][:, ds(128 * kt, 128)],
                       qT[h][:, ds(QC * qc + 256, 256)], start=True, stop=True)
                else:
                    mm(ps2[:, i, :], kT[h][:, ds(128 * kt, 128)],
                       qT[h][:, ds(QC * qc, QC)], start=True, stop=True)
            return ps2

        def emit_expmask(u, ps2):
            qc, h, g = u
            diagA = g == 2 * qc
            diagB = g == 2 * qc + 1
            ex = exp_.tile([128, 2, QC], F8)
            if diagB:
                nc.scalar.activation(ex[:, :, 256:512], ps2[:, :, 256:512],
                                     FX.Exp, scale=0.125 / (WSC * WSC),
                                     bias=sh[:])
                nc.gpsimd.affine_select(
                    out=ex[:, 0, 256:384], in_=ex[:, 0, 256:384],
                    compare_op=AL.is_ge, fill=0.0,
                    base=0, pattern=[[1, 128]], channel_multiplier=-1)
                nc.gpsimd.affine_select(
                    out=ex[:, 1, 256:512], in_=ex[:, 1, 256:512],
                    compare_op=AL.is_ge, fill=0.0,
                    base=-128, pattern=[[1, 256]], channel_multiplier=-1)
            else:
                nc.scalar.activation(ex[:], ps2[:], FX.Exp,
                                     scale=0.125 / (WSC * WSC), bias=sh[:])
                if diagA:
                    nc.gpsimd.affine_select(
                        out=ex[:, 0, 0:128], in_=ex[:, 0, 0:128],
                        compare_op=AL.is_ge, fill=0.0,
                        base=0, pattern=[[1, 128]], channel_multiplier=-1)
                    nc.gpsimd.affine_select(
                        out=ex[:, 1, 0:256], in_=ex[:, 1, 0:256],
                        compare_op=AL.is_ge, fill=0.0,
                        base=-128, pattern=[[1, 256]], channel_multiplier=-1)
            return ex

        def emit_pv(u, ex):
            qc, h, g = u
            ngr = 2 * qc + 2
            diagB = g == 2 * qc + 1
            if g == 0:
                ya = yap.tile([128, QC], F32, tag="ya", name=f"ya_{qc}_{h}")
                ya_tiles[(qc, h)] = ya
            ya = ya_tiles[(qc, h)]
            qsl = ds(256, 256) if diagB else ds(0, QC)
            for lohi, vt in enumerate((vhi, vlo)):
                mm(
                    ya[:, qsl],
                    vt[:, ds(2 * g, 2), ds(128 * h, 128)],
                    ex[:, :, qsl],
                    start=(g == 0 and lohi == 0),
                    stop=(g == ngr - 1 and lohi == 1),
                    perf_mode=DR,
                    skip_group_check=True,
                )

        def emit_norm(u):
            qc, h, g = u
            ya = ya_tiles.pop((qc, h))
            yc = ycp.tile([65, QC], F32, tag="yc", name=f"yc_{qc}_{h}")
            nc.vector.tensor_copy(yc[:], ya[0:65, :])
            rd = rdp.tile([65, QC], BF16)
            with nc.allow_low_precision(reason="bf16 softmax denom recip"):
                nc.vector.reciprocal(rd[64:65, :], yc[64:65, :])
            db = mmp.tile([128, QC], F32, tag="mm")
            mm(db[0:64, :], onesT[64:65, 0:64], rd[64:65, :], start=True,
               stop=True)
            if h == 0:
                dst = ytA[0:64, ds(QC * qc, QC)]
            elif h == 1:
                dst = ytA[64:128, ds(QC * qc, QC)]
            else:
                dst = ytB[0:64, ds(QC * qc, QC)]
            nc.vector.tensor_mul(dst, yc[0:64, :], db[0:64, :])

        # ---- prologue: minimum work before the first exp can start ----
        xT_load(0, 2 * QC)  # tokens for blocks 0-1
        nc.sync.dma_start(wsth[:, :, 0:256], wchr[:, :, 0:256])
        nc.sync.dma_start(wstl[:, :, 0:256], wclr[:, :, 0:256])
        nc.sync.dma_start(
            bcol[:, 0:4], bc[ds(0, 512)].rearrange("(m p) -> p m", p=128)
        )
        nc.sync.dma_start(bcol[0:64, 4:5], bc[ds(512, 64)])
        qkv_mtile(0, 0)
        qkv_mtile(0, 1)
        ps2_tiles = {0: emit_scores(units[0])}
        nc.sync.dma_start(wsth[:, :, 256:576], wchr[:, :, 256:576])
        nc.sync.dma_start(wstl[:, :, 256:576], wclr[:, :, 256:576])
        qkv_mtile(0, 2)
        qkv_mtile(0, 3)
        for tt in range(4):
            v_tile(tt)
        fillers.append((1400, lambda: qkv_mtile(0, 4)))
        fillers.append((0, lambda: xT_load(2 * QC, T)))  # rest of the tokens
        fillers.append((0, lambda: nc.sync.dma_start(wpA[:], wp[0:128, :])))
        fillers.append((0, lambda: nc.sync.dma_start(wpB[:], wp[128:193, :])))
        push_block_fillers(0)

        # ---- software-pipelined main loop ----
        # scores are emitted TWO pair-units ahead, so the Act engine always
        # has ~2 exp ops of runway queued while the PE works through scores,
        # PV, and drip-fed phase-1/proj filler between units.
        ps2_tiles[1] = emit_scores(units[1])
        fidx = 0.0
        cur_qc = 0
        nblock = len(fillers)
        for i, u in enumerate(units):
            if i + 2 < len(units):
                nq = units[i + 2][0]
                if nq != cur_qc:
                    # next block's qkv inputs must be emitted before its
                    # scores; stragglers only — most were pulled as filler
                    while fillers:
                        fillers.pop(0)[1]()
                    push_block_fillers(nq)
                    cur_qc = nq
                    fidx = 0.0
                    nblock = len(fillers)
                ps2_tiles[i + 2] = emit_scores(units[i + 2])
            ex = emit_expmask(u, ps2_tiles.pop(i))
            emit_pv(u, ex)
            if u[2] == 2 * u[0] + 1:  # last pair of (qc, h)
                emit_norm(u)
            # pull fillers at a steady rate across this qc's units
            npairs = HPC * (2 * cur_qc + 2)
            fidx += nblock / (0.9 * npairs)
            while fillers and fidx >= 1.0:
                fillers.pop(0)[1]()
                fidx -= 1.0
        while fillers:
            fillers.pop(0)[1]()
        for tt in range(4 * (NQC - 1), T // 128):
            proj_tile(tt, end=True)

    # ---- ReduceScatter over the batch's 4 cores, emit slice ----
    if collective:
        nc.gpsimd.collective_compute(
            "ReduceScatter",
            mybir.AluOpType.add,
            replica_groups=[[0, 1, 2, 3], [4, 5, 6, 7]],
            ins=[partial.opt()],
            outs=[rsout.opt()],
        )
        nc.sync.dma_start(outp[:], rsout[:])


_PROGRAM = None


def build_program(collective=True):
    global _PROGRAM
    if collective and _PROGRAM is not None:
        return _PROGRAM
    from contextlib import ExitStack

    nc = bacc.Bacc(
        trn_type="TRN2",
        target_bir_lowering=False,
        debug=False,
        num_devices=NCORES if collective else 1,
    )
    with tile.TileContext(nc) as tc:
        with ExitStack() as ctx:
            _body(ctx, tc, collective=collective)
    nc.compile()
    if collective:
        _PROGRAM = nc
    return nc


def make_in_maps(x, Wqkv, bqkv, Wproj, bproj):
    x = np.asarray(x, dtype=np.float32)
    Wqkv = np.asarray(Wqkv, dtype=np.float32)
    bqkv = np.asarray(bqkv, dtype=np.float32)
    Wproj = np.asarray(Wproj, dtype=np.float32)
    bproj = np.asarray(bproj, dtype=np.float32)
    bf = ml_dtypes.bfloat16
    f8 = ml_dtypes.float8_e4m3fn

    in_maps = []
    for c in range(NCORES):
        b, g = divmod(c, G)
        h = [3 * g + j for j in range(HPC)]  # global head ids
        qs = [Wqkv[:, 64 * hh : 64 * hh + 64] for hh in h]
        ks = [Wqkv[:, C + 64 * hh : C + 64 * hh + 64] for hh in h]
        vs = [Wqkv[:, 2 * C + 64 * hh : 2 * C + 64 * hh + 64] for hh in h]
        wcc = np.concatenate(
            [qs[0], qs[1], ks[0], ks[1], vs[0], vs[1], qs[2], vs[2], ks[2]], axis=1
        )
        bq = [bqkv[64 * hh : 64 * hh + 64] for hh in h]
        bk = [bqkv[C + 64 * hh : C + 64 * hh + 64] for hh in h]
        bv = [bqkv[2 * C + 64 * hh : 2 * C + 64 * hh + 64] for hh in h]
        bcc = np.concatenate(
            [bq[0], bq[1], bk[0], bk[1], bv[0], bv[1], bq[2], bv[2], bk[2]]
        )
        wprows = np.concatenate(
            [Wproj[64 * hh : 64 * hh + 64, :] for hh in h]
            + [(bproj if g == 0 else np.zeros_like(bproj))[None, :]],
            axis=0,
        )
        xb = np.ascontiguousarray(x[b]).astype(bf).astype(np.float32)
        xhi = xb.astype(f8)
        xlo = (xb - xhi.astype(np.float32)).astype(f8)
        xhl = (
            xhi.view(np.uint8).astype(np.uint16)
            | (xlo.view(np.uint8).astype(np.uint16) << 8)
        )
        ws = (wcc * 16.0).astype(np.float32)
        whi = ws.astype(f8)
        wlo = (ws - whi.astype(np.float32)).astype(f8)
        in_maps.append(
            {
                "xb": xhl,
                "wch": np.ascontiguousarray(whi),
                "wcl": np.ascontiguousarray(wlo),
                "bc": np.ascontiguousarray(bcc * 16.0),
                "wp": np.ascontiguousarray(wprows).astype(bf),
            }
        )
    return in_maps


def kernel(x, Wqkv, bqkv, Wproj, bproj):
    nc = build_program()
    in_maps = make_in_maps(x, Wqkv, bqkv, Wproj, bproj)
    res = run_bass_kernel_spmd(nc, in_maps, list(range(NCORES)))
    out = np.empty((2, T, C), dtype=np.float32)
    for c in range(NCORES):
        b, g = divmod(c, G)
        out[b, TSL * g : TSL * (g + 1), :] = res.results[c]["outp"]
    return out
